# revision 7
# baseline (speedup 1.0000x reference)
"""FNS spectral network kernel v2 for 8x TRN2 NeuronCores (1 sample/core).

Math per sample b (validated vs reference in fp64 numpy, rel err ~3e-7):
    rh = (-Gi) @ r @ Gi.T ; x = conv1..conv3 -> *theta -> conv4..conv6 ;
    e  = H @ x @ H.T

Device structure (big-tile layout, see prep2.py emulator):
  - X tiles [128, 43*260] fp16: block b data cols [260b+1, 260b+258), zero
    pads between. Rows: mids il1..6 at [(il-1)*16+cp], il0 halo [96:112],
    il7 halo [112:128] (cp = reim*8 + ch).
  - conv layer: per block 3 matmuls (dj taps = col shifts) with stationary
    T [128, 3*128] incl dup output cols ([96:112]=inn5, [112:128]=inn0),
    then ONE full-128 PSUM->SBUF eviction into the block's own col window
    (alternating DVE/ACT).
  - halo exchange per 11-block chunk via 3 SBUF->SBUF DMAs: dups -> S
    staging, then S shifted +-1 block into neighbor halo slots. Chunk c's
    dups are staged BEFORE chunk c-1's halo fill overwrites them.
  - T variants: z (block 0: il0 weights zeroed), b (block 42: il6/il7
    weights zeroed) make unfilled halo slots harmless.
  - conv3 emits psA (std) + psB (re/im-swapped) which DVE-multiplies with
    packed theta (signs baked in); GPSIMD adds -> x4 (std layout).
  - conv6: M=12 (re/im out), evicted per block to Y6 [12, 43*257], then 12
    partition-scatter DMAs build xo row-major planes for the back
    transform (dense matmuls as in v1).
"""

import os

import numpy as np

import concourse.bacc as bacc
import concourse.mybir as mybir
from concourse.bass_utils import run_bass_kernel_spmd
from concourse.tile import TileContext

F16 = mybir.dt.float16
F32 = mybir.dt.float32

B = 8
N1 = 255
CROP = 257
CH = 8
NBLK = 43
BSTR = 260
XW = NBLK * BSTR
CHUNKS = [(0, 11), (11, 11), (22, 11), (33, 10)]   # (start, count)
THW = 2 * CROP                                     # theta cols per block

LAST_EXEC_TIME_NS = None


# ----------------------------------------------------------------------------
# Host-side prep
# ----------------------------------------------------------------------------

def _host_consts():
    j = np.arange(CROP)[:, None]
    n = np.arange(N1)[None, :]
    Gi = (np.sin(np.pi * (j - 128) * (n + 1) / 256.0) / 256.0).astype(np.float32)
    k = np.arange(N1)[:, None]
    jj = np.arange(CROP)[None, :]
    H = np.exp(-2j * np.pi * k * (jj - 127.0) / 513.0)
    return {
        "g1t": np.ascontiguousarray((-Gi).T.astype(np.float16)),   # [255,257]
        "g2t": np.ascontiguousarray(Gi.T.astype(np.float16)),      # [255,257]
        "hrt": np.ascontiguousarray(H.real.T.astype(np.float16)),  # [257,255]
        "hit": np.ascontiguousarray(H.imag.T.astype(np.float16)),
        "hnit": np.ascontiguousarray((-H.imag).T.astype(np.float16)),
    }


def _expand_w(wre, wim):
    Co, Ci = wre.shape[0], wre.shape[1]
    W = np.zeros((2 * Co, 2 * Ci, 3, 3), np.float32)
    W[:Co, :Ci] = wre
    W[:Co, Ci:] = -wim
    W[Co:, :Ci] = wim
    W[Co:, Ci:] = wre
    return W


def _wT(wre, wim):
    return (np.swapaxes(np.swapaxes(wre, 0, 1), -2, -1),
            -np.swapaxes(np.swapaxes(wim, 0, 1), -2, -1))


def _row_std(p):
    if p < 96:
        return 1 + p // 16, p % 16
    if p < 112:
        return 0, p - 96
    return 7, p - 112


def _col_std_dup(m):
    if m < 96:
        return m // 16, m % 16
    if m < 112:
        return 5, m - 96
    return 0, m - 112


def _col_c6(m):
    return m % 6, m // 6


def _build_T(Wexp, rowmap, colmap, K, M, zero_hi=False, zero_lo=False):
    T = np.zeros((K, 3 * M), np.float32)
    Cin2 = Wexp.shape[1]
    for p in range(K):
        il, cp = rowmap(p)
        if cp >= Cin2:
            continue
        if zero_hi and il >= 6:
            continue
        if zero_lo and il == 0:
            continue
        for dj in range(3):
            for m in range(M):
                inn, op = colmap(m)
                di = il - inn
                if 0 <= di <= 2:
                    T[p, dj * M + m] = Wexp[op, cp, di, dj]
    return T.astype(np.float16)


def _host_prep_sample(bidx, inputs, consts):
    s = {}
    s["r16"] = np.ascontiguousarray(inputs["r"][bidx, 0].astype(np.float16))
    s.update(consts)

    w1 = (inputs["w1_re"][bidx], inputs["w1_im"][bidx])
    w2 = (inputs["w2_re"][bidx], inputs["w2_im"][bidx])
    w3 = (inputs["w3_re"][bidx], inputs["w3_im"][bidx])

    W1r = _expand_w(*w1)[:, 0:1]
    W2 = _expand_w(*w2)
    W3 = _expand_w(*w3)
    W3s = np.concatenate([W3[CH:], W3[:CH]], axis=0)
    W4 = _expand_w(*_wT(*w3))
    W5 = _expand_w(*_wT(*w2))
    W6 = _expand_w(*_wT(*w1))

    def row_x1(p):
        return p, 0

    s["t1"] = _build_T(W1r, row_x1, _col_std_dup, 8, 128)
    s["t1b"] = _build_T(W1r, row_x1, _col_std_dup, 8, 128, zero_hi=True)
    for key, W in (("t2", W2), ("t3", W3), ("t3s", W3s), ("t4", W4), ("t5", W5)):
        s[key] = _build_T(W, _row_std, _col_std_dup, 128, 128)
        s[key + "z"] = _build_T(W, _row_std, _col_std_dup, 128, 128, zero_lo=True)
        s[key + "b"] = _build_T(W, _row_std, _col_std_dup, 128, 128, zero_hi=True)
    s["t6"] = _build_T(W6, _row_std, _col_c6, 128, 12)
    s["t6z"] = _build_T(W6, _row_std, _col_c6, 128, 12, zero_lo=True)
    s["t6b"] = _build_T(W6, _row_std, _col_c6, 128, 12, zero_hi=True)

    # theta pack [128, NBLK*514]; sign baked: col0 block = +tr, col1 = -/+ti
    tr = inputs["theta_re"][bidx]
    ti = inputs["theta_im"][bidx]
    th = np.zeros((128, NBLK * THW), np.float16)
    for b in range(NBLK):
        base = b * THW
        ninn = 6 if b < NBLK - 1 else 5
        for p in range(128):
            if p < 96:
                inn, op = p // 16, p % 16
            elif p < 112:
                inn, op = 5, p - 96
            else:
                inn, op = 0, p - 112
            if inn >= ninn:
                continue
            row = 6 * b + inn
            ch = op % 8
            th[p, base:base + CROP] = tr[ch, row]
            th[p, base + CROP:base + THW] = (-ti[ch, row]) if op < 8 else ti[ch, row]
    s["thet"] = th
    return s


# ----------------------------------------------------------------------------
# Device program
# ----------------------------------------------------------------------------

def _build_nc():
    nc = bacc.Bacc(None, target_bir_lowering=False, debug=False)

    dp = {}
    decls = [("r16", [N1, N1]), ("g1t", [N1, CROP]), ("g2t", [N1, CROP]),
             ("hrt", [CROP, N1]), ("hit", [CROP, N1]), ("hnit", [CROP, N1]),
             ("t1", [8, 384]), ("t1b", [8, 384]),
             ("t6", [128, 36]), ("t6z", [128, 36]), ("t6b", [128, 36]),
             ("thet", [128, NBLK * THW])]
    for key in ("t2", "t3", "t3s", "t4", "t5"):
        for suf in ("", "z", "b"):
            decls.append((key + suf, [128, 384]))
    for name, shape in decls:
        dp[name] = nc.declare_dram_parameter(name, list(shape), F16,
                                             isOutput=False)
    ere = nc.declare_dram_parameter("ere", [N1, N1], F32, isOutput=True)
    eim = nc.declare_dram_parameter("eim", [N1, N1], F32, isOutput=True)
    dbg = {}
    if os.environ.get("K2_DEBUG"):
        for nm in ("dxa", "dxb", "dxc", "dx5", "dx6"):
            dbg[nm] = nc.declare_dram_parameter(nm, [128, XW], F16,
                                                isOutput=True)
        dbg["dxo"] = nc.declare_dram_parameter("dxo", [128, 3 * 2 * CROP],
                                               F16, isOutput=True)

    with TileContext(nc) as tc:
        with (
            tc.tile_pool(name="const", bufs=1) as pc,
            tc.tile_pool(name="xbuf", bufs=1) as px,
            tc.tile_pool(name="work", bufs=1) as pw,
            tc.tile_pool(name="wk2", bufs=3) as pw2,
            tc.tile_pool(name="psum", bufs=8, space="PSUM") as pp,
        ):
            # ---------------- constant loads ----------------
            def load_const(name, eng=None):
                shape = [int(x) for x in dp[name].shape]
                t = pc.tile(shape, F16, name=name, tag=name)
                (eng or nc.sync).dma_start(t[:, :], dp[name][:, :])
                return t

            def load_chunks(name, rows, cols, eng=None):
                out = []
                r0 = 0
                while r0 < rows:
                    rr = min(128, rows - r0)
                    t = pc.tile([rr, cols], F16, name=f"{name}{r0}",
                                tag=f"{name}{r0}")
                    (eng or nc.sync).dma_start(t[:, :], dp[name][r0:r0 + rr, :])
                    out.append(t)
                    r0 += rr
                return out

            r_sb = load_chunks("r16", N1, N1)
            g1_sb = load_chunks("g1t", N1, CROP)
            g2_sb = load_chunks("g2t", N1, CROP)
            hr_sb = load_chunks("hrt", CROP, N1, nc.scalar)
            hi_sb = load_chunks("hit", CROP, N1, nc.scalar)
            hn_sb = load_chunks("hnit", CROP, N1, nc.scalar)
            tsb = {}
            for name, shape in decls:
                if name.startswith("t") and name != "thet":
                    tsb[name] = load_const(name, nc.scalar)

            def tvar(key, b):
                if b == 0 and key + "z" in tsb:
                    return tsb[key + "z"]
                if b == NBLK - 1:
                    if key + "b" in tsb:
                        return tsb[key + "b"]
                return tsb[key]

            # ---------------- big X tiles + pads ----------------
            x1 = px.tile([8, XW], F16, name="x1", tag="x1")
            XA = px.tile([128, XW], F16, name="XA", tag="XA")
            XB = px.tile([128, XW], F16, name="XB", tag="XB")
            XC = px.tile([128, XW], F16, name="XC", tag="XC")
            y6 = px.tile([12, NBLK * CROP], F16, name="y6", tag="y6")

            for X in (x1, XA, XB, XC):
                v = X[:, :].rearrange("p (b c) -> p b c", c=BSTR)
                nc.vector.memset(v[:, :, 0:1], 0.0)
                nc.vector.memset(v[:, :, 258:260], 0.0)
            nc.vector.memset(x1[0:1, 0:BSTR], 0.0)
            nc.vector.memset(x1[0:8, BSTR * 42:], 0.0)
            for X in (XA, XB, XC):
                nc.vector.memset(X[96:112, 0:BSTR], 0.0)
                nc.vector.memset(X[96:128, BSTR * 42:], 0.0)

            # ---------------- front transform ----------------
            vt_sb = [pw.tile([128, CROP], F16, name="vt0", tag="vt0"),
                     pw.tile([127, CROP], F16, name="vt1", tag="vt1")]
            for m, (m0, mm) in enumerate(((0, 128), (128, 127))):
                ps = pp.tile([128, CROP], F32, name="ps", tag="ps", bufs=3)
                for k2 in range(2):
                    nc.tensor.matmul(
                        ps[0:mm, :], lhsT=r_sb[k2][:, m0:m0 + mm],
                        rhs=g1_sb[k2][:, :], start=(k2 == 0), stop=(k2 == 1))
                nc.scalar.copy(vt_sb[m][:, :], ps[0:mm, :])

            rh_sb = [pw.tile([128, CROP], F16, name="rh0", tag="rh0"),
                     pw.tile([128, CROP], F16, name="rh1", tag="rh1"),
                     pw.tile([1, CROP], F16, name="rh2", tag="rh2")]
            for m, (m0, mm) in enumerate(((0, 128), (128, 128), (256, 1))):
                ps = pp.tile([128, CROP], F32, name="ps", tag="ps", bufs=3)
                for k2 in range(2):
                    nc.tensor.matmul(
                        ps[0:mm, :], lhsT=vt_sb[k2][:, m0:m0 + mm],
                        rhs=g2_sb[k2][:, :], start=(k2 == 0), stop=(k2 == 1))
                nc.vector.tensor_copy(rh_sb[m][:, :], ps[0:mm, :])

            # x1 scatter: rows 6b-1..6b+6 -> x1[0:8, block b window], per
            # block 1-2 contiguous-partition DMAs (v1-proven plain APs).
            for b in range(NBLK):
                lo = max(0, 6 * b - 1)
                hi = min(256, 6 * b + 6)
                r0 = lo
                while r0 <= hi:
                    c = r0 // 128
                    c_end = min(hi, c * 128 + 127)
                    cnt = c_end - r0 + 1
                    il0 = r0 - (6 * b - 1)
                    (nc.sync if b % 2 else nc.scalar).dma_start(
                        x1[il0:il0 + cnt, BSTR * b + 1:BSTR * b + 258],
                        rh_sb[c][r0 - c * 128:r0 - c * 128 + cnt, :])
                    r0 = c_end + 1

            # ---------------- conv machinery ----------------
            S_W = 11 * CROP

            def strips_stage(XO, c, cstart, cnt, Svar):
                """DMA1: stage chunk c's dups into S."""
                xv = XO[:, :].rearrange("p (b c) -> p b c", c=BSTR)
                sv = Svar[:, :].rearrange("p (b c) -> p b c", c=CROP)
                nc.sync.dma_start(sv[:, 0:cnt, :],
                                  xv[96:128, cstart:cstart + cnt, 1:258])

            def strips_fill_main(XO, cstart, cnt, Svar):
                """halo fill whose dests stay clear of the NEXT chunk's
                un-staged dups: right-dir covers blocks cstart+1..cstart+cnt-1
                only; left-dir is always safe (prev chunk already staged)."""
                xv = XO[:, :].rearrange("p (b c) -> p b c", c=BSTR)
                sv = Svar[:, :].rearrange("p (b c) -> p b c", c=CROP)
                nb2 = cnt - 1
                if nb2 > 0:
                    nc.gpsimd.dma_start(
                        xv[96:112, cstart + 1:cstart + 1 + nb2, 1:258],
                        sv[0:16, 0:nb2, :])
                o = 1 if cstart == 0 else 0
                nb3 = cnt - o
                if nb3 > 0:
                    nc.scalar.dma_start(
                        xv[112:128, cstart + o - 1:cstart + o - 1 + nb3, 1:258],
                        sv[16:32, o:o + nb3, :])

            def strips_fill_cross(XO, cstart, cnt, Svar):
                """the one right-dir dest crossing into the next chunk's first
                block; emitted after that chunk's dups are staged."""
                if cstart + cnt >= NBLK:
                    return
                xv = XO[:, :].rearrange("p (b c) -> p b c", c=BSTR)
                sv = Svar[:, :].rearrange("p (b c) -> p b c", c=CROP)
                nc.gpsimd.dma_start(
                    xv[96:112, cstart + cnt:cstart + cnt + 1, 1:258],
                    sv[0:16, cnt - 1:cnt, :])

            def conv_layer(tkey, XI, XO, kin):
                prevS = None
                for ci, (cstart, cnt) in enumerate(CHUNKS):
                    for b in range(cstart, cstart + cnt):
                        t = tvar(tkey, b)
                        ps = pp.tile([128, CROP], F32, name="ps", tag="ps", bufs=3)
                        for dj in range(3):
                            nc.tensor.matmul(
                                ps[:, :], lhsT=t[:, dj * 128:(dj + 1) * 128],
                                rhs=XI[0:kin, BSTR * b + dj:BSTR * b + dj + CROP],
                                start=(dj == 0), stop=(dj == 2))
                        dst = XO[:, BSTR * b + 1:BSTR * b + 1 + CROP]
                        if b % 2 == 0:
                            nc.vector.tensor_copy(dst, ps[:, :])
                        else:
                            nc.scalar.copy(dst, ps[:, :])
                    Svar = pw2.tile([32, S_W], F16, name="S", tag="S", bufs=3)
                    strips_stage(XO, ci, cstart, cnt, Svar)
                    if prevS is not None:
                        strips_fill_cross(XO, *prevS)
                    strips_fill_main(XO, cstart, cnt, Svar)
                    prevS = (cstart, cnt, Svar)

            conv_layer("t1", x1, XA, 8)
            if dbg:
                nc.sync.dma_start(dbg["dxa"][:, :], XA[:, :])
            conv_layer("t2", XA, XB, 128)
            if dbg:
                nc.sync.dma_start(dbg["dxb"][:, :], XB[:, :])

            # conv3 + theta: XB -> XC
            prevS = None
            for ci, (cstart, cnt) in enumerate(CHUNKS):
                tht = pw2.tile([128, 11 * THW], F16, name="tht", tag="tht",
                               bufs=2)
                nc.gpsimd.dma_start(
                    tht[:, 0:cnt * THW],
                    dp["thet"][:, cstart * THW:(cstart + cnt) * THW])
                for b in range(cstart, cstart + cnt):
                    tA = tvar("t3", b)
                    tB = tvar("t3s", b)
                    psA = pp.tile([128, CROP], F32, name="psA", tag="psA", bufs=2)
                    psB = pp.tile([128, CROP], F32, name="psB", tag="psB", bufs=2)
                    for dj in range(3):
                        rhs = XB[:, BSTR * b + dj:BSTR * b + dj + CROP]
                        nc.tensor.matmul(
                            psA[:, :], lhsT=tA[:, dj * 128:(dj + 1) * 128],
                            rhs=rhs, start=(dj == 0), stop=(dj == 2))
                    for dj in range(3):
                        rhs = XB[:, BSTR * b + dj:BSTR * b + dj + CROP]
                        nc.tensor.matmul(
                            psB[:, :], lhsT=tB[:, dj * 128:(dj + 1) * 128],
                            rhs=rhs, start=(dj == 0), stop=(dj == 2))
                    tb = (b - cstart) * THW
                    u = pw2.tile([128, CROP], F16, name="u", tag="u", bufs=4)
                    v = pw2.tile([128, CROP], F16, name="v", tag="v", bufs=4)
                    nc.vector.tensor_mul(u[:, :], psA[:, :],
                                         tht[:, tb:tb + CROP])
                    nc.vector.tensor_mul(v[:, :], psB[:, :],
                                         tht[:, tb + CROP:tb + THW])
                    nc.gpsimd.tensor_add(
                        XC[:, BSTR * b + 1:BSTR * b + 1 + CROP],
                        u[:, :], v[:, :])
                Svar = pw2.tile([32, S_W], F16, name="S", tag="S", bufs=3)
                strips_stage(XC, ci, cstart, cnt, Svar)
                if prevS is not None:
                    strips_fill_cross(XC, *prevS)
                strips_fill_main(XC, cstart, cnt, Svar)
                prevS = (cstart, cnt, Svar)

            if dbg:
                nc.sync.dma_start(dbg["dxc"][:, :], XC[:, :])
            conv_layer("t4", XC, XA, 128)
            if dbg:
                nc.sync.dma_start(dbg["dx5"][:, :], XA[:, :])
            conv_layer("t5", XA, XB, 128)
            if dbg:
                nc.sync.dma_start(dbg["dx6"][:, :], XB[:, :])

            # conv6: XB -> y6 [12, NBLK*CROP]
            for b in range(NBLK):
                t = tvar("t6", b)
                ps = pp.tile([128, CROP], F32, name="ps", tag="ps", bufs=3)
                for dj in range(3):
                    nc.tensor.matmul(
                        ps[0:12, :], lhsT=t[:, dj * 12:(dj + 1) * 12],
                        rhs=XB[:, BSTR * b + dj:BSTR * b + dj + CROP],
                        start=(dj == 0), stop=(dj == 2))
                dst = y6[:, CROP * b:CROP * (b + 1)]
                if b % 2 == 0:
                    nc.vector.tensor_copy(dst, ps[0:12, :])
                else:
                    nc.scalar.copy(dst, ps[0:12, :])

            # scatter y6 -> xoc chunks [128, 514] (re cols [0:257], im [257:514])
            # per block: src [12,257] viewed [(inn 2p),(op 1p),(col)] ->
            # dest rows 6b.. viewed [6, 2, 257]; chunk-crossing blocks split.
            xoc = [pw.tile([128, 2 * CROP], F16, name="xoc0", tag="xoc0"),
                   pw.tile([128, 2 * CROP], F16, name="xoc1", tag="xoc1"),
                   pw.tile([1, 2 * CROP], F16, name="xoc2", tag="xoc2")]
            for b in range(NBLK):
                pieces = []
                ninn = 6 if b < NBLK - 1 else 5
                i0 = 0
                while i0 < ninn:
                    r = 6 * b + i0
                    c = r // 128
                    csz = 128 if c < 2 else 1
                    iend = min(ninn - 1, (c * 128 + csz - 1 - 6 * b))
                    pieces.append((c, i0, iend - i0 + 1))
                    i0 = iend + 1
                for (c, i0, ni) in pieces:
                    p0 = 6 * b + i0 - 128 * c
                    for op in range(2):
                        eng = (nc.sync, nc.scalar, nc.gpsimd)[(2 * b + op) % 3]
                        eng.dma_start(
                            xoc[c][p0:p0 + ni, op * CROP:op * CROP + CROP],
                            y6[6 * op + i0:6 * op + i0 + ni,
                               CROP * b:CROP * (b + 1)])

            if dbg:
                for ci in range(2):
                    nc.sync.dma_start(
                        dbg["dxo"][:, 2 * CROP * ci:2 * CROP * (ci + 1)],
                        xoc[ci][:, :])
                nc.sync.dma_start(dbg["dxo"][0:1, 4 * CROP:6 * CROP],
                                  xoc[2][:, :])

            # ---------------- back transform ----------------
            at = {}
            for p in ("re", "im"):
                at[p] = [pw.tile([128, N1], F16, name=f"at{p}0", tag=f"at{p}0"),
                         pw.tile([128, N1], F16, name=f"at{p}1", tag=f"at{p}1"),
                         pw.tile([1, N1], F16, name=f"at{p}2", tag=f"at{p}2")]
            for m, (m0, mm) in enumerate(((0, 128), (128, 128), (256, 1))):
                for p, terms in (("re", ((0, hr_sb), (1, hn_sb))),
                                 ("im", ((0, hi_sb), (1, hr_sb)))):
                    ps = pp.tile([128, N1], F32, name="ps", tag="ps", bufs=3)
                    nmm = 0
                    for (xi, hsb) in terms:
                        for k2 in range(3):
                            nc.tensor.matmul(
                                ps[0:mm, :],
                                lhsT=xoc[k2][:, xi * CROP + m0:
                                             xi * CROP + m0 + mm],
                                rhs=hsb[k2][:, :],
                                start=(nmm == 0), stop=(nmm == 5))
                            nmm += 1
                    nc.scalar.copy(at[p][m][:, :], ps[0:mm, :])

            e_sb = {}
            for p in ("re", "im"):
                e_sb[p] = [pw.tile([128, N1], F32, name=f"e{p}0", tag=f"e{p}0"),
                           pw.tile([127, N1], F32, name=f"e{p}1", tag=f"e{p}1")]
            for m, (m0, mm) in enumerate(((0, 128), (128, 127))):
                for p, terms in (("re", (("re", hr_sb), ("im", hn_sb))),
                                 ("im", (("re", hi_sb), ("im", hr_sb)))):
                    ps = pp.tile([128, N1], F32, name="ps", tag="ps", bufs=3)
                    nmm = 0
                    for (ap_, hsb) in terms:
                        for k2 in range(3):
                            nc.tensor.matmul(
                                ps[0:mm, :],
                                lhsT=at[ap_][k2][:, m0:m0 + mm],
                                rhs=hsb[k2][:, :],
                                start=(nmm == 0), stop=(nmm == 5))
                            nmm += 1
                    nc.vector.tensor_copy(e_sb[p][m][:, :], ps[0:mm, :])

            for p, dram in (("re", ere), ("im", eim)):
                nc.sync.dma_start(dram[0:128, :], e_sb[p][0][:, :])
                nc.sync.dma_start(dram[128:255, :], e_sb[p][1][:, :])

    nc.finalize()
    return nc


_NC_CACHE = None


def _get_nc():
    global _NC_CACHE
    if _NC_CACHE is None:
        _NC_CACHE = _build_nc()
    return _NC_CACHE


def kernel(**inputs):
    global LAST_EXEC_TIME_NS
    inputs = {k: np.asarray(v) for k, v in inputs.items()}
    consts = _host_consts()
    in_maps = [_host_prep_sample(b, inputs, consts) for b in range(B)]
    nc = _get_nc()
    trace = bool(os.environ.get("BASS_TRACE"))
    res = run_bass_kernel_spmd(nc, in_maps, list(range(B)), trace=trace)
    LAST_EXEC_TIME_NS = res.exec_time_ns
    out = np.zeros((B, 1, N1, N1), np.complex64)
    for b in range(B):
        out[b, 0] = res.results[b]["ere"] + 1j * res.results[b]["eim"]
    return out
